# revision 18
# baseline (speedup 1.0000x reference)
"""Trainium2 Bass kernel for nn_LinearEncoder (gnn_message_passing), v6.

Reference, for N=512 nodes, n_in = n_out = 256:
    i, j = triu_indices(N, k=1)
    h = concat([x[i], x[j]]) @ W.T + b        # [E, 256]
    out[i, j] = h ; out = out + out.T         # [512, 512, 256], 0 diagonal

Algebraic identity (W = [W1 | W2]):  out[i, j] = A[min] + B'[max],
    A = x @ W1.T,  B' = x @ W2.T + b,  zero diagonal.

Exploits output symmetry: each unordered pair {r, j} is computed on ONE
core as bf16 and mirrored by the host.  Row r owns circular distances
d = 1..255 (+256 for r < 256); core k (rows [64k, 64k+64)) needs rotated
cols s in [t+1, t+256] per local row t.  v6 trims the padded cover at
16-column granularity: the 8-row chunk c needs only 17 of 20 16-wide
segments (s in [16*(c//2), 16*(c//2)+272)) — 8.9 MB/core out.

Device program (partition dim = output channel):
  - PE: transposed tables AT[ch, s] = W1 @ xT_rot, BpT = W2 @ xT_rot + b
    (bf16 in/out; extra rounding is dwarfed by the bf16 output).
  - mix (input rmask = R_s): M8[(scol, t8)] = (AT + R*(BpT-AT)) x8;
    BSEG[ch, (j, t)] = BpT - R_j*(BpT-AT) at 64- then 16-granularity.
  - per 8-row chunk: slab[ch, (j, s, t)] = M8-window + BSEG16 broadcast
    over s — one DVE tensor_tensor in the 2x_1P perf mode ((j, s, t)
    layout keeps stride-1 bf16 pairs innermost on all operands).
  - sync queue streams h=0 chunks, scalar queue h=1 chunks to HBM.
"""

import os
import sys

for _p in ("/opt/trn_rl_repo", "/root/.axon_site/_ro/trn_rl_repo"):
    if os.path.isdir(_p) and _p not in sys.path:
        sys.path.insert(0, _p)

import numpy as np
import ml_dtypes

import concourse.bass as bass
import concourse.bacc as bacc
import concourse.mybir as mybir
import concourse.tile as tile
from concourse.bass_utils import run_bass_kernel_spmd

N = 512
CH = 256          # n_out
NIN = 256         # n_in
NCORES = 8
RB = N // NCORES  # 64 rows per core
SCOL = 320        # rotated-column rectangle width
NSEG = 5          # 64-wide column segments
NS16 = 20         # 16-wide column segments
J17 = 17          # 16-wide segments per trimmed chunk
CW = J17 * 16 * 8  # chunk free width (2176)
F32 = mybir.dt.float32
BF16 = mybir.dt.bfloat16
BF16NP = ml_dtypes.bfloat16


# --------------------------------------------------------------------------
# host-side input builder
# --------------------------------------------------------------------------

def _core_inputs(x, W, b, k):
    x = np.asarray(x, np.float32)
    W = np.asarray(W, np.float32)
    b = np.asarray(b, np.float32)
    base = RB * k
    idx = (base + np.arange(SCOL)) % N
    xr = np.ascontiguousarray(x.T[:, idx])        # [f, s] rotated
    wrap = N - base
    seg_r = np.array([1.0 if 64 * (j + 1) <= wrap else 0.0
                      for j in range(NSEG)], np.float32)
    rmask = np.broadcast_to(np.repeat(seg_r, 64), (128, SCOL))
    return {
        "xh": xr.astype(BF16NP),
        "rmask": np.ascontiguousarray(rmask).astype(BF16NP),
        "wa": np.ascontiguousarray(W[:, :NIN].T).astype(BF16NP),
        "wb": np.ascontiguousarray(W[:, NIN:].T).astype(BF16NP),
        "bcol": b.reshape(1, CH).astype(BF16NP),
    }


# --------------------------------------------------------------------------
# device program
# --------------------------------------------------------------------------

_PROGRAM = None


def _build_program() -> bass.Bass:
    nc = bacc.Bacc()
    AL = mybir.AluOpType

    d_xh = nc.dram_tensor("xh", [NIN, SCOL], BF16, kind="ExternalInput")
    d_wa = nc.dram_tensor("wa", [NIN, CH], BF16, kind="ExternalInput")
    d_wb = nc.dram_tensor("wb", [NIN, CH], BF16, kind="ExternalInput")
    d_bcol = nc.dram_tensor("bcol", [1, CH], BF16, kind="ExternalInput")
    d_rm = nc.dram_tensor("rmask", [128, SCOL], BF16, kind="ExternalInput")

    # outp[h][p, (c, jj, s, t)]: ch = 128h + p, row t' = 8c + t,
    # rotated col = 16*(c//2 + jj) + s.
    d_out = nc.dram_tensor("outp", [2, 128, 8 * CW], BF16,
                           kind="ExternalOutput")

    with tile.TileContext(nc) as tc:
        with (
            tc.tile_pool(name="const", bufs=1) as cpool,
            tc.tile_pool(name="ps", bufs=4, space="PSUM") as ps,
            tc.tile_pool(name="slab0", bufs=6) as sp0,
            tc.tile_pool(name="slab1", bufs=6) as sp1,
        ):
            def load(dram, shape, dtype, tag, eng=None):
                t = cpool.tile(shape, dtype, tag=tag, name=tag)
                (eng or nc.sync).dma_start(out=t[:], in_=dram)
                return t

            xh0 = load(d_xh[0:128, :], [128, SCOL], BF16, "xh0")
            wa0 = load(d_wa[0:128, :], [128, CH], BF16, "wa0")
            xh1 = load(d_xh[128:256, :], [128, SCOL], BF16, "xh1",
                       nc.scalar)
            wa1 = load(d_wa[128:256, :], [128, CH], BF16, "wa1", nc.scalar)
            wb0 = load(d_wb[0:128, :], [128, CH], BF16, "wb0")
            wb1 = load(d_wb[128:256, :], [128, CH], BF16, "wb1", nc.scalar)
            bcol = load(d_bcol[:], [1, CH], BF16, "bcol")
            rmt = load(d_rm[:], [128, SCOL], BF16, "rmt", nc.scalar)

            ones = cpool.tile([1, SCOL], BF16, tag="ones", name="ones")
            nc.vector.memset(ones[:], 1.0)

            M8, BS16 = {}, {}

            def prep(h):
                cs = slice(128 * h, 128 * (h + 1))
                tabs = {}
                for nm, w0, w1, with_b in (("A", wa0, wa1, False),
                                           ("B", wb0, wb1, True)):
                    p = ps.tile([128, SCOL], F32, tag="pt",
                                name=f"pt{nm}{h}")
                    mm = nc.tensor.matmul
                    mm(p[:], w0[:, cs], xh0[:], start=True, stop=False)
                    mm(p[:], w1[:, cs], xh1[:], start=False,
                       stop=not with_b)
                    if with_b:
                        mm(p[:], bcol[0:1, cs], ones[:], start=False,
                           stop=True)
                    t = cpool.tile([128, SCOL], BF16, tag=f"T{nm}{h}",
                                   name=f"T{nm}{h}")
                    nc.scalar.copy(out=t[:], in_=p[:])
                    tabs[nm] = t
                AT, BpT = tabs["A"], tabs["B"]

                d = cpool.tile([128, SCOL], BF16, tag=f"d{h}", name=f"d{h}")
                nc.vector.tensor_sub(d[:], BpT[:], AT[:])
                tm = cpool.tile([128, SCOL], BF16, tag=f"tm{h}",
                                name=f"tm{h}")
                nc.vector.tensor_mul(tm[:], d[:], rmt[:])

                # Tile allocations keep the proven SBUF order (m8, bs, tb)
                # — only the instruction order changes: BSEG ops first so
                # the ACT-side BS16 replication overlaps the M8 build.
                m8 = cpool.tile([128, 8 * SCOL], BF16, tag=f"M8{h}",
                                name=f"M8{h}")
                m8v = m8[:].rearrange("p (s t) -> p s t", t=8)
                bs = cpool.tile([128, NSEG * RB], BF16, tag=f"BS{h}",
                                name=f"BS{h}")
                tb = cpool.tile([128, NSEG * RB], BF16, tag=f"tb{h}",
                                name=f"tb{h}")

                # BSEG64[(j5, t)] = BpT[t] - R_j * d[t]
                jt = lambda ap: (ap[:, 0:RB].unsqueeze(1)
                                 .broadcast_to([128, NSEG, RB]))
                tbv = tb[:].rearrange("p (j t) -> p j t", j=NSEG)
                nc.vector.tensor_mul(
                    tbv, jt(d), rmt[:].rearrange("p (j t) -> p j t",
                                                 j=NSEG))
                nc.vector.tensor_sub(
                    bs[:].rearrange("p (j t) -> p j t", j=NSEG),
                    jt(BpT), tbv)

                # M8[(scol, t8)] = 8 copies of AT + R*d  (col = 8*scol + t)
                nc.vector.tensor_add(m8v[:, :, 0:1].squeeze(2),
                                     tm[:], AT[:])
                dbl = nc.vector if h == 0 else nc.scalar
                for w in (1, 2, 4):
                    if dbl is nc.scalar:
                        dbl.copy(out=m8v[:, :, w:2 * w],
                                 in_=m8v[:, :, 0:w])
                    else:
                        dbl.tensor_copy(out=m8v[:, :, w:2 * w],
                                        in_=m8v[:, :, 0:w])
                b16 = cpool.tile([128, NS16 * RB], BF16, tag=f"B16{h}",
                                 name=f"B16{h}")
                nc.scalar.copy(
                    out=b16[:].rearrange("p (j q t) -> p j q t",
                                         j=NSEG, q=4),
                    in_=(bs[:].rearrange("p (j t) -> p j t", j=NSEG)
                         .unsqueeze(2).broadcast_to([128, NSEG, 4, RB])))
                M8[h], BS16[h] = m8, b16

            def chunk(c, h, half=None):
                # half=0/1: 4-row sub-chunk (smaller final DMA drain)
                j0 = c // 2
                nt = 8 if half is None else 4
                t0 = 8 * c + (0 if not half else 4)
                w = J17 * 16 * nt
                off = CW * c + (0 if not half else w)
                pool = sp0 if h == 0 else sp1
                slab = pool.tile([128, w], BF16, tag="sl",
                                 name=f"sl{c}_{h}_{half}")
                sh = [128, J17, 16, nt]
                out_ap = slab[:].rearrange(
                    "p (j s t) -> p j s t", j=J17, s=16, t=nt)
                m_ap = M8[h][:, 128 * j0:128 * j0 + CW].rearrange(
                    "p (j s t) -> p j s t", j=J17, s=16, t=8)
                if half is not None:
                    m_ap = m_ap[:, :, :, 4 * half:4 * half + 4]
                b_ap = (BS16[h][:]
                        .rearrange("p (j t) -> p j t", j=NS16)
                        [:, j0:j0 + J17, t0:t0 + nt]
                        .unsqueeze(2).broadcast_to(sh))
                nc.vector.tensor_tensor(out_ap, m_ap, b_ap, AL.add)
                q = nc.sync if h == 0 else nc.scalar
                q.dma_start(out=d_out[h][:, off:off + w], in_=slab[:])

            prep(0)
            chunk(0, 0, 0)
            chunk(0, 0, 1)
            chunk(1, 0)
            prep(1)
            chunk(0, 1, 0)
            chunk(0, 1, 1)
            for c in range(2, 7):
                chunk(c, 0)
                chunk(c - 1, 1)
            # tail: finish both queues with alternating 4-row halves so
            # neither drains a full chunk after the last TT
            chunk(6, 1)
            chunk(7, 0, 0)
            chunk(7, 1, 0)
            chunk(7, 0, 1)
            chunk(7, 1, 1)

    nc.compile()
    return nc


def _program() -> bass.Bass:
    global _PROGRAM
    if _PROGRAM is None:
        _PROGRAM = _build_program()
    return _PROGRAM


# --------------------------------------------------------------------------
# host entry point
# --------------------------------------------------------------------------

_IDX = {}


def _band_idx(dmax):
    """(t_idx, s_idx) of rectangle entries with 1 <= s - t <= dmax."""
    if dmax not in _IDX:
        t, s = np.mgrid[0:RB, 0:SCOL]
        m = (s - t >= 1) & (s - t <= dmax)
        _IDX[dmax] = (t[m], s[m])
    return _IDX[dmax]


def _rot_slab(v):
    """outp array [2, 128, 8*CW] -> rotated slab [RB, SCOL, CH].

    Chunks 0-6 are [J17, 16, 8]; chunk 7 is stored as two 4-row halves.
    """
    slab = np.zeros((RB, SCOL, CH), np.float32)

    def place(c, rows, w):                        # w: [2, 128, J17, 16, nt]
        s0 = 16 * (c // 2)
        blk = w.transpose(4, 2, 3, 0, 1).reshape(
            len(rows), J17 * 16, CH)               # t' scol (h ch)
        slab[rows[0]:rows[0] + len(rows), s0:s0 + J17 * 16] = blk

    hw = J17 * 16 * 4
    for c in range(8):
        if c in (0, 7):
            for half in (0, 1):
                off = CW * c + hw * half
                place(c, range(8 * c + 4 * half, 8 * c + 4 * half + 4),
                      v[:, :, off:off + hw].reshape(2, 128, J17, 16, 4))
        else:
            place(c, range(8 * c, 8 * c + 8),
                  v[:, :, CW * c:CW * (c + 1)].reshape(2, 128, J17, 16, 8))
    return slab


def _assemble(results):
    out = np.zeros((N * N, CH), np.float32)
    for k in range(NCORES):
        base = RB * k
        slab = _rot_slab(np.asarray(results[k]["outp"])
                         .astype(np.float32))
        t_idx, s_idx = _band_idx(256 if k < 4 else 255)
        r_idx = base + t_idx
        j_idx = (base + s_idx) % N
        vals = slab[t_idx, s_idx]
        out[r_idx * N + j_idx] = vals
        out[j_idx * N + r_idx] = vals
    return out.reshape(N, N, CH)


def build_in_maps(x, W, b):
    return [_core_inputs(x, W, b, k) for k in range(NCORES)]


def kernel(x, W, b):
    nc = _program()
    in_maps = build_in_maps(x, W, b)
    res = run_bass_kernel_spmd(nc, in_maps, core_ids=list(range(NCORES)))
    return _assemble(res.results)
